# revision 28
# baseline (speedup 1.0000x reference)
"""Trainium2 Bass kernel for nn_BinaryDiceLoss_blobPunish.

Reference semantics:
  pmask = predict > max(predict)/2 ; tmask likewise
  labels = 200 iterations of (3x3 maxpool * mask) label propagation, seeds = arange
  num_label_blobs = count_unique(labels) - 1 ; num_target_blobs = count_unique(tlabels)
  pen = clip(sqrt(nl/nt), 1, 16) ; dice_b = 1 - (sum(p*t)+1)/(sum(p^2+t^2)+1)
  loss = mean(dice) * pen

Sharding: data-parallel over batch; core c handles predict+target images
{2c, 2c+1}. Everything stays SBUF-resident: 128 partitions = 4 images x 32
partition-groups, 16 image rows per partition (slots of 514 cols = 512 real
+ 2 zero pads for the horizontal 3-max window).

Active kernel (build_program_v3, chosen because this execution backend has a
large fixed per-instruction cost, so instruction count dominates):
  op1/op2: horizontal 3-max as two pair-max scalar_tensor_tensor ops
  ghost fill: 2 SBUF->SBUF DMAs copy each partition's neighbor rows into
     ghost slots 0/17 of the H buffer; image blocks are ordered by DESCENDING
     seed base (target seeds shifted +16*2^18, an order-isomorphic relabeling
     that preserves the unique count), which makes every bottom-ghost
     cross-image leak lose the max(); a third partition-strided DMA zeroes
     the 3 top-boundary ghost rows from a zero tile.
  op3/op4: vertical 3-max as two pair-maxes over ghosted slots (op4 in place)
  op5: L' = V * mask
Global threshold via one AllReduce(max); broadcast back with a ones-matmul.
Unique count: device marks "candidate" pixels (label differs from all
W/NW/N/NE raster-prior neighbors; every first occurrence of a distinct value
is provably a candidate), host np.unique's the returned candidate values
(per-image label ranges are disjoint). Dice/mask sums ride along as fused
per-partition accumulator outputs.

build_program (v1) is the PE-shift-matmul variant kept as a validated
fallback; sync_chain=True adds the per-op semaphore chains CoreSim's race
detector requires (hardware orders same-engine DVE ops via the pipeline
drain, so the chains are dropped on hardware builds).
"""

import numpy as np

N_CORES = 8
NIT = 200
S = 16          # row slots per partition
W = 514         # slot width (512 real cols + 2 zero pads)
RW = 512
PK = S * RW     # packed free size (8192)
FREE = S * W    # padded free size (8224)
HW_IMG = 512 * 512

_PROG_CACHE = {}


def _build_perms():
    pu = np.zeros((128, 128), dtype=np.float32)  # out[p] = in[p+1]
    pd = np.zeros((128, 128), dtype=np.float32)  # out[p] = in[p-1]
    for p in range(127):
        if p not in (31, 63, 95):
            pu[p + 1, p] = 1.0
    for p in range(1, 128):
        if p not in (32, 64, 96):
            pd[p - 1, p] = 1.0
    return pu, pd


def build_program(nit=NIT, use_collective=True, debug_dump=False, sync_chain=True):
    import concourse.bass as bass
    import concourse.mybir as mybir
    import concourse.bass_isa as bass_isa

    f32 = mybir.dt.float32
    Alu = mybir.AluOpType

    nc = bass.Bass(num_devices=N_CORES, detect_race_conditions=sync_chain)
    ld_out = (nc.declare_dram_parameter("ldump", [128, FREE], f32, isOutput=True)
              if debug_dump else None)

    x_in = nc.declare_dram_parameter("x", [128, S, RW], f32, isOutput=False)
    cb_in = nc.declare_dram_parameter("cbase", [128, 1], f32, isOutput=False)
    pu_in = nc.declare_dram_parameter("permup", [128, 128], f32, isOutput=False)
    pd_in = nc.declare_dram_parameter("permdn", [128, 128], f32, isOutput=False)
    thr_in = nc.declare_dram_parameter("thrin", [128, 1], f32, isOutput=False)
    st_out = nc.declare_dram_parameter("stats", [128, 8], f32, isOutput=True)
    cv_out = nc.declare_dram_parameter("cand", [128, PK], f32, isOutput=True)

    ccin = nc.dram_tensor("ccin", [2], f32)
    ccout = nc.dram_tensor("ccout", [2], f32, addr_space="Shared")

    core_ids = list(range(N_CORES))

    from contextlib import ExitStack
    with ExitStack() as ctx:
        P1 = ctx.enter_context(nc.sbuf_tensor([128, FREE], f32))
        P2 = ctx.enter_context(nc.sbuf_tensor([128, FREE], f32))
        T1 = ctx.enter_context(nc.sbuf_tensor([128, FREE], f32))
        T2 = ctx.enter_context(nc.sbuf_tensor([128, FREE], f32))
        MK = ctx.enter_context(nc.sbuf_tensor([128, PK], f32))
        THR = ctx.enter_context(nc.sbuf_tensor([128, 1], f32))
        PMX = ctx.enter_context(nc.sbuf_tensor([128, 1], f32))
        STATS = ctx.enter_context(nc.sbuf_tensor([128, 8], f32))
        CB = ctx.enter_context(nc.sbuf_tensor([128, 1], f32))
        PU = ctx.enter_context(nc.sbuf_tensor([128, 128], f32))
        PD = ctx.enter_context(nc.sbuf_tensor([128, 128], f32))
        PSH = ctx.enter_context(nc.psum_tensor([128, RW], f32))
        PSB = ctx.enter_context(nc.psum_tensor([128, RW], f32))
        PSL = ctx.enter_context(nc.psum_tensor([128, W], f32))
        PSC = ctx.enter_context(nc.psum_tensor([128, 2], f32))
        SCR = ctx.enter_context(nc.sbuf_tensor([128, 128], f32))
        SC2 = ctx.enter_context(nc.sbuf_tensor([128, 2], f32))
        ONE = ctx.enter_context(nc.sbuf_tensor([128, 128], f32))
        s_load = ctx.enter_context(nc.semaphore("s_load"))
        s_hd = ctx.enter_context(nc.semaphore("s_hd"))
        s_bd = ctx.enter_context(nc.semaphore("s_bd"))
        s_ps = ctx.enter_context(nc.semaphore("s_ps"))
        s_cc = ctx.enter_context(nc.semaphore("s_cc"))
        s_gp = ctx.enter_context(nc.semaphore("s_gp"))
        s_gdma = ctx.enter_context(nc.semaphore("s_gdma"))
        s_dv = ctx.enter_context(nc.semaphore("s_dv"))
        s_out = ctx.enter_context(nc.semaphore("s_out"))
        s_vch = ctx.enter_context(nc.semaphore("s_vch"))
        s_cp = ctx.enter_context(nc.semaphore("s_cp"))
        s_lx = ctx.enter_context(nc.semaphore("s_lx"))
        s_th = ctx.enter_context(nc.semaphore("s_th"))
        s_pb = ctx.enter_context(nc.semaphore("s_pb"))
        s_gch = ctx.enter_context(nc.semaphore("s_gch"))
        block = ctx.enter_context(nc.Block())

        class _Chain:
            """Orders same-engine ops for the race detector with exactly one
            sem update per instruction. An op may substitute its own update
            (sem, value-after-inc); the next op waits on whatever the
            previous op updated. With sync_chain off (hardware: the DVE
            drain already orders same-engine ops) only the substituted
            updates are emitted."""
            def __init__(self, sem, on=True):
                self.sem = sem
                self.n = 0
                self.last = None
                self.on = on

            def __call__(self, ins, update=None):
                if self.on and self.last is not None:
                    ins._wait_ge(*self.last)
                if update is None:
                    if self.on:
                        self.n += 1
                        ins.then_inc(self.sem, 1)
                        self.last = (self.sem, self.n)
                else:
                    sem, val = update
                    ins.then_inc(sem, 1)
                    self.last = (sem, val)
                return ins
        def pview(buf):  # padded [p, slot, col(514)] view
            return buf.ap().rearrange("p (s w) -> p s w", w=W)

        def pkview(buf):  # packed [p, slot, col(512)] view of first PK elems
            return buf[:, 0:PK].rearrange("p (s c) -> p s c", c=RW)

        P1v, P2v, T1v, T2v = pview(P1), pview(P2), pview(T1), pview(T2)
        MKv = MK.ap().rearrange("p (s c) -> p s c", c=RW)
        n_loads = 3  # cbase + permup + permdn  (+1 if thrin); x loads on s_lx
        if not use_collective:
            n_loads += 1

        @block.sync
        def _(sync):
            q = PK // 4
            for i in range(4):
                sync.dma_start(
                    out=T1[:, i * q:(i + 1) * q], in_=x_in.ap().rearrange(
                        "p s c -> p (s c)")[:, i * q:(i + 1) * q]
                ).then_inc(s_lx, 16)
            sync.dma_start(out=CB[:], in_=cb_in[:]).then_inc(s_load, 16)
            sync.dma_start(out=PU[:], in_=pu_in[:]).then_inc(s_load, 16)
            sync.dma_start(out=PD[:], in_=pd_in[:]).then_inc(s_load, 16)
            if not use_collective:
                sync.dma_start(out=THR[:], in_=thr_in[:]).then_inc(s_load, 16)
            # stage target half at base partition 0 for the dice product
            # (scalar_tensor_tensor requires equal base partitions for SBUF ins)
            sync.wait_ge(s_lx, 64)
            sync.dma_start(out=P2[0:64, 0:PK], in_=T1[64:128, 0:PK]
                           ).then_inc(s_cp, 16)
            # outputs
            sync.wait_ge(s_dv, 1)
            sync.dma_start(out=st_out[:], in_=STATS[:]).then_inc(s_out, 16)
            sync.dma_start(out=cv_out[:], in_=T2[:, 0:PK]).then_inc(s_out, 16)
            n_out = 32
            if debug_dump:
                sync.dma_start(out=ld_out[:], in_=P1[:]).then_inc(s_out, 16)
                n_out += 16
            sync.wait_ge(s_out, n_out)

        @block.gpsimd
        def _(gpsimd):
            # seeds-iota (f32-exact: values < 2^21): val = 8192*p + 512*s + c + 1
            gpsimd.iota(
                T2[:, 0:PK], pattern=[[RW, S], [1, RW]], base=1,
                channel_multiplier=PK,
                allow_small_or_imprecise_dtypes=True,
            ).then_inc(s_gp, 1)
            if use_collective:
                gpsimd.wait_ge(s_th, 1)  # PMX ready
                # repartition per-partition maxes into partition 0
                gpsimd.dma_start(out=SCR[0:1, 0:128], in_=PMX[:, 0]
                                 ).then_inc(s_gdma, 16)
                gpsimd.wait_ge(s_th, 2)  # SC2 = [local pmax, local tmax]
                gpsimd.dma_start(out=ccin[:], in_=SC2[0:1, 0:2]
                                 ).then_inc(s_gdma, 16)
                gpsimd.wait_ge(s_gdma, 32)
                gpsimd.collective_compute(
                    "AllReduce", Alu.max, replica_groups=[core_ids],
                    ins=[ccin[:]], outs=[ccout[:]],
                ).then_inc(s_cc, 1)
                gpsimd.wait_ge(s_cc, 1)
                gpsimd.dma_start(out=SC2[0:1, 0:2], in_=ccout[:]
                                 ).then_inc(s_gdma, 16)
                gpsimd.wait_ge(s_gdma, 48)
                # signal SC2 holds global [pmax, tmax]
                gpsimd.nop().then_inc(s_gp, 1)

        @block.tensor
        def _(tensor):
            tensor.wait_ge(s_load, n_loads * 16)
            tensor.wait_ge(s_lx, 64)
            if use_collective:
                # broadcast SC2[0, 0:2] to all partitions via ones-matmul
                tensor.wait_ge(s_th, 1)  # ONE memset done (via DVE chain)
                tensor.wait_ge(s_gp, 2)  # SC2 holds global maxes
                tensor.matmul(PSC[:, 0:2], lhsT=ONE[0:1, 0:128],
                              rhs=SC2[0:1, 0:2], start=True, stop=True
                              ).then_inc(s_pb, 1)
            for k in range(nit):
                mm1 = tensor.matmul(PSH[:, 0:RW], lhsT=PU[:],
                                    rhs=T2v[:, 0, 1:513],
                                    start=True, stop=True)
                mm1._wait_ge(s_hd, k + 1)
                mm1.then_inc(s_ps, 1)
                mm2 = tensor.matmul(PSB[:, 0:RW], lhsT=PD[:],
                                    rhs=T1v[:, 15, 1:513],
                                    start=True, stop=True)
                mm2._wait_ge(s_bd, k + 1)
                mm2.then_inc(s_ps, 1)
            # row-above of final labels, full padded width (cols 0..513)
            tensor.wait_ge(s_hd, nit + 1)
            tensor.matmul(PSL[:, 0:RW], lhsT=PD[:], rhs=P1v[:, 15, 0:512],
                             start=True, stop=True).then_inc(s_ps, 1)
            tensor.matmul(PSL[:, RW:W], lhsT=PD[:], rhs=P1v[:, 15, 512:514],
                             start=True, stop=True).then_inc(s_ps, 1)

        @block.vector
        def _(vector):
            V = _Chain(s_vch, sync_chain)

            def stt(update=None, prewait=None, **kw):
                if prewait is not None and sync_chain:
                    vector.wait_ge(s_ps, prewait)
                ins = vector.scalar_tensor_tensor(**kw)
                if prewait is not None and not sync_chain:
                    ins._wait_ge(s_ps, prewait)
                return V(ins, update)
            V(vector.memset(P1[:], 0.0))
            V(vector.memset(STATS[:], 0.0))
            V(vector.memset(ONE[0:1, 0:128], 1.0))
            vector.wait_ge(s_load, n_loads * 16)
            vector.wait_ge(s_lx, 64)
            # per-partition max of raw x
            V(vector.tensor_reduce(
                PMX[:, 0:1], T1[:, 0:PK], axis=mybir.AxisListType.X,
                op=Alu.max), update=(s_th, 1))
            # dice numerators: sum(p*t) per partition (predict rows 0:64)
            vector.wait_ge(s_cp, 16)
            stt(out=MK[0:64, 0:PK], in0=T1[0:64, 0:PK], scalar=0.0,
                in1=P2[0:64, 0:PK], op0=Alu.bypass, op1=Alu.mult,
                accum_out=STATS[0:64, 0:1])
            # sum of squares per partition
            stt(out=P2[:, 0:PK], in0=T1[:, 0:PK], scalar=0.0,
                in1=T1[:, 0:PK], op0=Alu.bypass, op1=Alu.mult,
                accum_out=STATS[:, 1:2])
            V(vector.tensor_copy(out=STATS[:, 3:4], in_=PMX[:, 0:1]))
            if use_collective:
                vector.wait_ge(s_gdma, 16)  # SCR[0, 0:128] = partition maxes
                V(vector.tensor_reduce(
                    SC2[0:1, 0:1], SCR[0:1, 0:64], axis=mybir.AxisListType.X,
                    op=Alu.max))
                V(vector.tensor_reduce(
                    SC2[0:1, 1:2], SCR[0:1, 64:128], axis=mybir.AxisListType.X,
                    op=Alu.max), update=(s_th, 2))
                vector.wait_ge(s_pb, 1)  # PSC = broadcast global maxes
                V(vector.tensor_scalar(out=THR[0:64, 0:1], in0=PSC[0:64, 0:1],
                                       scalar1=0.5, scalar2=None, op0=Alu.mult))
                V(vector.tensor_scalar(out=THR[64:128, 0:1],
                                       in0=PSC[64:128, 1:2],
                                       scalar1=0.5, scalar2=None, op0=Alu.mult))
            # mask + per-partition mask sums
            V(vector.tensor_scalar(out=MK[:], in0=T1[:, 0:PK],
                                   scalar1=THR[:, 0:1], scalar2=None,
                                   op0=Alu.is_gt, op1=Alu.add,
                                   accum_out=STATS[:, 2:3]))
            # seeds -> L0 (into padded P1): (iota + cbase) * mask
            vector.wait_ge(s_gp, 1)
            stt(out=P1v[:, :, 1:513], in0=T2[:, 0:PK], scalar=CB[:, 0:1],
                in1=MK[:], op0=Alu.add, op1=Alu.mult)
            V(vector.memset(P2[:], 0.0))

            def pswait(k2, ins=None):
                # chained builds carry a chain wait on every op, so the PSUM
                # wait must be standalone; bare builds attach it to the op.
                if sync_chain or ins is None:
                    vector.wait_ge(s_ps, k2)
                    return ins
                return ins._wait_ge(s_ps, k2)

            for k in range(nit):
                src_, dst = (P1v, P2v) if k % 2 == 0 else (P2v, P1v)
                op1 = stt(out=T1v[:, :, 0:513], in0=src_[:, :, 0:513],
                          scalar=0.0, in1=src_[:, :, 1:514], op0=Alu.max,
                          op1=Alu.max, prewait=(2 * k if k > 0 else None))
                stt(out=T2v[:, :, 1:513], in0=T1v[:, :, 0:512], scalar=0.0,
                    in1=T1v[:, :, 1:513], op0=Alu.max, op1=Alu.max,
                    update=(s_hd, k + 1))
                stt(out=T1v[:, 0:15, 1:513], in0=T2v[:, 0:15, 1:513], scalar=0.0,
                    in1=T2v[:, 1:16, 1:513], op0=Alu.max, op1=Alu.max)
                stt(out=T1v[:, 15:16, 1:513], in0=T2v[:, 15:16, 1:513], scalar=0.0,
                    in1=PSH[:, 0:RW].rearrange("p (s c) -> p s c", c=RW),
                    op0=Alu.max, op1=Alu.max, update=(s_bd, k + 1),
                    prewait=2 * k + 1)
                stt(out=T2v[:, 1:16, 1:513], in0=T1v[:, 0:15, 1:513], scalar=0.0,
                    in1=T1v[:, 1:16, 1:513], op0=Alu.max, op1=Alu.max)
                stt(out=T2v[:, 0:1, 1:513], in0=T1v[:, 0:1, 1:513], scalar=0.0,
                    in1=PSB[:, 0:RW].rearrange("p (s c) -> p s c", c=RW),
                    op0=Alu.max, op1=Alu.max, prewait=2 * k + 2)
                if k == nit - 1:
                    stt(out=dst[:, :, 1:513], in0=T2v[:, :, 1:513], scalar=0.0,
                        in1=MKv[:, :, :], op0=Alu.max, op1=Alu.mult,
                        update=(s_hd, nit + 1))
                else:
                    stt(out=dst[:, :, 1:513], in0=T2v[:, :, 1:513], scalar=0.0,
                        in1=MKv[:, :, :], op0=Alu.max, op1=Alu.mult)


            def tt(**kw):
                return V(vector.tensor_tensor(**kw))
            q1, q2, q3, q4 = pkview(T1), pkview(T2), pkview(P2), MKv
            L = P1v
            PSLv = PSL.ap()
            # W: L[c] vs L[c-1] (pad col 0 handles image edge)
            tt(out=q1[:, :, :], in0=L[:, :, 1:513], in1=L[:, :, 0:512],
               op=Alu.not_equal)
            # N / NW / NE for slots 1..15 from SBUF
            tt(out=q2[:, 1:16, :], in0=L[:, 1:16, 1:513], in1=L[:, 0:15, 1:513],
               op=Alu.not_equal)
            vector.wait_ge(s_ps, 2 * nit + 2)
            tt(out=q2[:, 0:1, :], in0=L[:, 0:1, 1:513],
               in1=PSLv[:, 1:513].rearrange("p (s c) -> p s c", c=RW),
               op=Alu.not_equal)
            tt(out=q3[:, :, :], in0=q1[:, :, :], in1=q2[:, :, :], op=Alu.mult)
            # q1 <- NW, q2 <- NE
            tt(out=q1[:, 1:16, :], in0=L[:, 1:16, 1:513], in1=L[:, 0:15, 0:512],
               op=Alu.not_equal)
            tt(out=q1[:, 0:1, :], in0=L[:, 0:1, 1:513],
               in1=PSLv[:, 0:512].rearrange("p (s c) -> p s c", c=RW),
               op=Alu.not_equal)
            tt(out=q2[:, 1:16, :], in0=L[:, 1:16, 1:513], in1=L[:, 0:15, 2:514],
               op=Alu.not_equal)
            tt(out=q2[:, 0:1, :], in0=L[:, 0:1, 1:513],
               in1=PSLv[:, 2:514].rearrange("p (s c) -> p s c", c=RW),
               op=Alu.not_equal)
            # qq = (W*N) * (NW*NE) -> q1 ; cand = qq * L -> q2 (=T2, DMA'd out)
            tt(out=q4[:, :, :], in0=q1[:, :, :], in1=q2[:, :, :], op=Alu.mult)
            tt(out=q1[:, :, :], in0=q3[:, :, :], in1=q4[:, :, :], op=Alu.mult)
            V(vector.tensor_tensor(out=q2[:, :, :], in0=q1[:, :, :],
                                   in1=L[:, :, 1:513], op=Alu.mult),
              update=(s_dv, 1))

    return nc


def _host_combine(stats, cands):
    """stats: list of [128,8]; cands: list of [128, PK] per core."""
    dices = []
    for c in range(N_CORES):
        st = stats[c].astype(np.float64)
        for h in range(2):
            num = st[32 * h:32 * h + 32, 0].sum()
            den = (st[32 * h:32 * h + 32, 1].sum()
                   + st[64 + 32 * h:64 + 32 * h + 32, 1].sum())
            dices.append(1.0 - (num + 1.0) / (den + 1.0))
    mdice = np.mean(dices)

    pv = np.concatenate([cands[c][0:64].ravel() for c in range(N_CORES)])
    tv = np.concatenate([cands[c][64:128].ravel() for c in range(N_CORES)])
    pu = np.unique(pv[pv > 0]).size
    tu = np.unique(tv[tv > 0]).size
    pm = sum(stats[c][0:64, 2].sum(dtype=np.float64) for c in range(N_CORES))
    tm = sum(stats[c][64:128, 2].sum(dtype=np.float64) for c in range(N_CORES))
    total = 16 * HW_IMG
    nl = pu + (1 if pm < total else 0) - 1
    nt = tu + (1 if tm < total else 0)
    nl_f = np.float32(nl)
    nt_f = np.float32(nt)
    with np.errstate(divide="ignore", invalid="ignore"):
        pen = np.sqrt(nl_f / nt_f)
    if not np.isfinite(pen):
        pen = np.float32(16.0)
    pen = np.clip(pen, np.float32(1.0), np.float32(16.0))
    return np.float32(np.float32(mdice) * pen)


def make_in_maps(predict, target):
    pu, pd = _build_perms()
    in_maps = []
    for c in range(N_CORES):
        xc = np.concatenate(
            [predict[2 * c:2 * c + 2, 0], target[2 * c:2 * c + 2, 0]], axis=0
        ).astype(np.float32)  # [4, 512, 512]
        xc = np.ascontiguousarray(xc.reshape(4, 32, 16, 512).reshape(128, 16, 512))
        cb = np.full((128, 1), 2 * c * HW_IMG, dtype=np.float32)
        cb[64:128] -= 2 * HW_IMG
        in_maps.append({
            "x": xc, "cbase": cb, "permup": pu, "permdn": pd,
            "thrin": np.zeros((128, 1), dtype=np.float32),
        })
    return in_maps


def kernel(predict, target):
    from concourse.bass_utils import run_bass_kernel_spmd

    key = "main"
    if key not in _PROG_CACHE:
        _PROG_CACHE[key] = build_program_v3(NIT, use_collective=True,
                                            sync_chain=False)
    nc = _PROG_CACHE[key]

    in_maps = make_in_maps3(np.asarray(predict), np.asarray(target))
    res = run_bass_kernel_spmd(nc, in_maps, list(range(N_CORES))).results
    stats = [res[c]["stats"] for c in range(N_CORES)]
    cands = [res[c]["cand"] for c in range(N_CORES)]
    return _host_combine3(stats, cands)


def build_program_v3(nit=NIT, use_collective=True, debug_dump=False,
                     sync_chain=True):
    """Instruction-count-minimized variant (this backend has a large fixed
    per-instruction cost and element count is nearly free): per iteration
      pool_h: 3-col overlapping-window max  L -> HG slots 1..16
      2 ghost DMAs: HG slot0[p] <- HG slot16[p-1], slot17[p] <- HG slot1[p+1]
      2 strided memsets re-zero the 4 image-boundary ghosts
      pool_v: 3-slot overlapping-window max HG slots 0..17 -> V
      stt:    L' = V * mask
    """
    import concourse.bass as bass
    import concourse.mybir as mybir

    f32 = mybir.dt.float32
    Alu = mybir.AluOpType

    nc = bass.Bass(num_devices=N_CORES, detect_race_conditions=sync_chain)
    ld_out = (nc.declare_dram_parameter("ldump", [128, FREE], f32, isOutput=True)
              if debug_dump else None)
    x_in = nc.declare_dram_parameter("x", [128, S, RW], f32, isOutput=False)
    cb_in = nc.declare_dram_parameter("cbase", [128, 1], f32, isOutput=False)
    nc.declare_dram_parameter("permup", [128, 128], f32, isOutput=False)
    nc.declare_dram_parameter("permdn", [128, 128], f32, isOutput=False)
    thr_in = nc.declare_dram_parameter("thrin", [128, 1], f32, isOutput=False)
    st_out = nc.declare_dram_parameter("stats", [128, 8], f32, isOutput=True)
    cv_out = nc.declare_dram_parameter("cand", [128, PK], f32, isOutput=True)
    ccin = nc.dram_tensor("ccin", [2], f32)
    ccout = nc.dram_tensor("ccout", [2], f32, addr_space="Shared")
    core_ids = list(range(N_CORES))
    HGF = 18 * RW
    LGO = 16 * RW + 128   # L-ghost region inside HG free range (514 wide)

    from contextlib import ExitStack
    with ExitStack() as ctx:
        P1 = ctx.enter_context(nc.sbuf_tensor([128, FREE], f32))
        P2 = ctx.enter_context(nc.sbuf_tensor([128, FREE], f32))
        HG = ctx.enter_context(nc.sbuf_tensor([128, HGF], f32))
        T2 = ctx.enter_context(nc.sbuf_tensor([128, FREE], f32))
        MK = ctx.enter_context(nc.sbuf_tensor([128, PK], f32))
        THR = ctx.enter_context(nc.sbuf_tensor([128, 1], f32))
        PMX = ctx.enter_context(nc.sbuf_tensor([128, 1], f32))
        STATS = ctx.enter_context(nc.sbuf_tensor([128, 8], f32))
        CB = ctx.enter_context(nc.sbuf_tensor([128, 1], f32))
        SCR = ctx.enter_context(nc.sbuf_tensor([128, 128], f32))
        SC2 = ctx.enter_context(nc.sbuf_tensor([128, 2], f32))
        ONE = ctx.enter_context(nc.sbuf_tensor([128, 128], f32))
        ZB = ctx.enter_context(nc.sbuf_tensor([128, RW], f32))
        PSC = ctx.enter_context(nc.psum_tensor([128, 2], f32))
        s_load = ctx.enter_context(nc.semaphore("s_load"))
        s_lx = ctx.enter_context(nc.semaphore("s_lx"))
        s_hd = ctx.enter_context(nc.semaphore("s_hd"))
        s_g1 = ctx.enter_context(nc.semaphore("s_g1"))
        s_gz = ctx.enter_context(nc.semaphore("s_gz"))
        s_gh = ctx.enter_context(nc.semaphore("s_gh"))
        s_cc = ctx.enter_context(nc.semaphore("s_cc"))
        s_gp = ctx.enter_context(nc.semaphore("s_gp"))
        s_gdma = ctx.enter_context(nc.semaphore("s_gdma"))
        s_dv = ctx.enter_context(nc.semaphore("s_dv"))
        s_out = ctx.enter_context(nc.semaphore("s_out"))
        s_th = ctx.enter_context(nc.semaphore("s_th"))
        s_pb = ctx.enter_context(nc.semaphore("s_pb"))
        s_cp = ctx.enter_context(nc.semaphore("s_cp"))
        s_vch = ctx.enter_context(nc.semaphore("s_vch"))
        block = ctx.enter_context(nc.Block())

        class _Chain:
            def __init__(self, sem, on):
                self.sem, self.n, self.last, self.on = sem, 0, None, on

            def __call__(self, ins, update=None):
                if self.on and self.last is not None:
                    ins._wait_ge(*self.last)
                if update is None:
                    if self.on:
                        self.n += 1
                        ins.then_inc(self.sem, 1)
                        self.last = (self.sem, self.n)
                else:
                    sem, val = update
                    ins.then_inc(sem, 1)
                    self.last = (sem, val)
                return ins

        def pview(buf):
            return buf.ap().rearrange("p (s w) -> p s w", w=W)

        P1v, P2v = pview(P1), pview(P2)
        HGv = HG.ap().rearrange("p (s c) -> p s c", c=RW)
        T2v = T2[:, 0:PK].rearrange("p (s c) -> p s c", c=RW)
        MKv = MK.ap().rearrange("p (s c) -> p s c", c=RW)

        def winh(buf):  # [p, s, c, 3] over padded L cols 0..513
            a = buf.ap()
            return bass.AP(tensor=a.tensor, offset=a.offset,
                           ap=[list(a.ap[0]), [W, S], [1, RW], [1, 3]])

        def winv():  # [p, r, c, 3] over HG slots 0..17
            a = HG.ap()
            return bass.AP(tensor=a.tensor, offset=a.offset,
                           ap=[list(a.ap[0]), [RW, S], [1, RW], [RW, 3]])

        def lg(d):  # L-ghost cols (1+d)..(512+d) as [p, 1, 512]
            return HG.ap()[:, LGO + 1 + d:LGO + 513 + d].rearrange(
                "p (s c) -> p s c", c=RW)

        n_loads = 1 + (0 if use_collective else 1)

        @block.sync
        def _(sync):
            q = PK // 4
            for i in range(4):
                sync.dma_start(
                    out=T2[:, i * q:(i + 1) * q], in_=x_in.ap().rearrange(
                        "p s c -> p (s c)")[:, i * q:(i + 1) * q]
                ).then_inc(s_lx, 16)
            sync.dma_start(out=CB[:], in_=cb_in[:]).then_inc(s_load, 16)
            if not use_collective:
                sync.dma_start(out=THR[:], in_=thr_in[:]).then_inc(s_load, 16)
            sync.wait_ge(s_lx, 64)
            sync.dma_start(out=MK[64:128, 0:PK], in_=T2[0:64, 0:PK]
                           ).then_inc(s_cp, 16)
            sync.wait_ge(s_dv, 1)
            sync.dma_start(out=st_out[:], in_=STATS[:]).then_inc(s_out, 16)
            sync.dma_start(out=cv_out[:], in_=T2[:, 0:PK]).then_inc(s_out, 16)
            n_out = 32
            if debug_dump:
                sync.dma_start(out=ld_out[:], in_=P1[:]).then_inc(s_out, 16)
                n_out += 16
            sync.wait_ge(s_out, n_out)

        @block.gpsimd
        def _(gpsimd):
            gpsimd.iota(
                P2[:, 0:PK], pattern=[[RW, S], [1, RW]], base=1,
                channel_multiplier=PK,
                allow_small_or_imprecise_dtypes=True,
            ).then_inc(s_gp, 1)
            if use_collective:
                gpsimd.wait_ge(s_th, 1)
                gpsimd.dma_start(out=SCR[0:1, 0:128], in_=PMX[:, 0]
                                 ).then_inc(s_gdma, 16)
                gpsimd.wait_ge(s_th, 2)
                gpsimd.dma_start(out=ccin[:], in_=SC2[0:1, 0:2]
                                 ).then_inc(s_gdma, 16)
                gpsimd.wait_ge(s_gdma, 32)
                gpsimd.collective_compute(
                    "AllReduce", Alu.max, replica_groups=[core_ids],
                    ins=[ccin[:]], outs=[ccout[:]],
                ).then_inc(s_cc, 1)
                gpsimd.wait_ge(s_cc, 1)
                gpsimd.dma_start(out=SC2[0:1, 0:2], in_=ccout[:]
                                 ).then_inc(s_gdma, 16)
                gpsimd.wait_ge(s_gdma, 48)
                gpsimd.nop().then_inc(s_gp, 1)
            for k in range(nit):
                # top ghosts: image blocks are ordered by DESCENDING seed base,
                # so bottom-ghost cross-image leaks lose every max() and only
                # the three top-boundary rows (p=32,64,96) need zeroing.
                g1 = gpsimd.dma_start(out=HGv[1:128, 0:1, :],
                                      in_=HGv[0:127, 16:17, :])
                g1._wait_ge(s_hd, k + 1)
                g1.then_inc(s_g1, 16)
                g2 = gpsimd.dma_start(out=HGv[0:127, 17:18, :],
                                      in_=HGv[1:128, 1:2, :])
                g2._wait_ge(s_hd, k + 1)
                g2.then_inc(s_g1, 16)
                z1 = gpsimd.dma_start(out=HGv[32:128:32, 0:1, :],
                                      in_=ZB[0:3, :])
                z1._wait_ge(s_g1, 32 * (k + 1))
                z1.then_inc(s_gz, 16)
            gl = gpsimd.dma_start(out=HG.ap()[1:128, LGO:LGO + 514],
                                  in_=P1.ap()[0:127, 15 * W:16 * W])
            gl._wait_ge(s_hd, nit + 1)
            gl.then_inc(s_gh, 16)

        @block.tensor
        def _(tensor):
            if use_collective:
                tensor.wait_ge(s_th, 1)
                tensor.wait_ge(s_gp, 2)
                tensor.matmul(PSC[:, 0:2], lhsT=ONE[0:1, 0:128],
                              rhs=SC2[0:1, 0:2], start=True, stop=True
                              ).then_inc(s_pb, 1)

        @block.vector
        def _(vector):
            V = _Chain(s_vch, sync_chain)

            def stt(update=None, **kw):
                return V(vector.scalar_tensor_tensor(**kw), update)

            V(vector.memset(P1[:], 0.0))
            V(vector.memset(STATS[:], 0.0))
            V(vector.memset(ONE[0:1, 0:128], 1.0))
            V(vector.memset(HG[:], 0.0))
            vector.wait_ge(s_load, n_loads * 16)
            vector.wait_ge(s_lx, 64)
            V(vector.tensor_reduce(
                PMX[:, 0:1], T2[:, 0:PK], axis=mybir.AxisListType.X,
                op=Alu.max), update=(s_th, 1))
            vector.wait_ge(s_cp, 16)
            stt(out=HG[64:128, 0:PK], in0=T2[64:128, 0:PK], scalar=0.0,
                in1=MK[64:128, 0:PK], op0=Alu.bypass, op1=Alu.mult,
                accum_out=STATS[64:128, 0:1])
            stt(out=HG[:, 0:PK], in0=T2[:, 0:PK], scalar=0.0,
                in1=T2[:, 0:PK], op0=Alu.bypass, op1=Alu.mult,
                accum_out=STATS[:, 1:2])
            V(vector.tensor_copy(out=STATS[:, 3:4], in_=PMX[:, 0:1]))
            if use_collective:
                vector.wait_ge(s_gdma, 16)
                V(vector.tensor_reduce(
                    SC2[0:1, 0:1], SCR[0:1, 0:64], axis=mybir.AxisListType.X,
                    op=Alu.max))
                V(vector.tensor_reduce(
                    SC2[0:1, 1:2], SCR[0:1, 64:128], axis=mybir.AxisListType.X,
                    op=Alu.max), update=(s_th, 2))
                vector.wait_ge(s_pb, 1)
                V(vector.tensor_scalar(out=THR[0:64, 0:1], in0=PSC[0:64, 0:1],
                                       scalar1=0.5, scalar2=None, op0=Alu.mult))
                V(vector.tensor_scalar(out=THR[64:128, 0:1],
                                       in0=PSC[64:128, 1:2],
                                       scalar1=0.5, scalar2=None, op0=Alu.mult))
            V(vector.tensor_scalar(out=MK[:], in0=T2[:, 0:PK],
                                   scalar1=THR[:, 0:1], scalar2=None,
                                   op0=Alu.is_gt, op1=Alu.add,
                                   accum_out=STATS[:, 2:3]))
            vector.wait_ge(s_gp, 1)
            stt(out=P1v[:, :, 1:513], in0=P2[:, 0:PK], scalar=CB[:, 0:1],
                in1=MK[:], op0=Alu.add, op1=Alu.mult)
            V(vector.memset(P2[:], 0.0))
            # re-zero HG (stats junk landed in slots 0..15)
            V(vector.memset(HG[:], 0.0))

            V(vector.memset(ZB[:], 0.0))
            T2a = pview(T2)

            def w(sem, val, **kw):
                # chained builds: standalone wait (chain wait occupies the
                # instruction slot); bare builds: attach to the op.
                if sync_chain:
                    vector.wait_ge(sem, val)
                ins = vector.scalar_tensor_tensor(**kw)
                if not sync_chain:
                    ins._wait_ge(sem, val)
                return ins

            for k in range(nit):
                srcv = P1v if k % 2 == 0 else P2v
                dstv = P2v if k % 2 == 0 else P1v
                stt(out=T2a[:, :, 0:513], in0=srcv[:, :, 0:513], scalar=0.0,
                    in1=srcv[:, :, 1:514], op0=Alu.max, op1=Alu.max)
                stt(out=HGv[:, 1:17, :], in0=T2a[:, :, 0:512], scalar=0.0,
                    in1=T2a[:, :, 1:513], op0=Alu.max, op1=Alu.max,
                    update=(s_hd, k + 1))
                # s_gz implies z1 done, which implies both ghost DMAs done
                V(w(s_gz, 16 * (k + 1), out=T2[:, 0:PK],
                    in0=HGv[:, 0:16, :], scalar=0.0, in1=HGv[:, 1:17, :],
                    op0=Alu.max, op1=Alu.max))
                stt(out=T2[:, 0:PK], in0=T2[:, 0:PK], scalar=0.0,
                    in1=HGv[:, 2:18, :], op0=Alu.max, op1=Alu.max)
                stt(out=dstv[:, :, 1:513], in0=T2[:, 0:PK], scalar=0.0,
                    in1=MK[:], op0=Alu.max, op1=Alu.mult)

            # memset L-ghost then signal gl DMA; final labels in P1
            V(vector.memset(HG.ap()[:, LGO:LGO + 514], 0.0),
              update=(s_hd, nit + 1))

            def tt(**kw):
                return V(vector.tensor_tensor(**kw))
            L = P1v
            q1 = HGv
            q2, q3, q4 = T2v, P2v, MKv
            # zero the cross-image L-ghost rows the gl DMA scribbled
            vector.wait_ge(s_gh, 16)
            tt(out=q1[:, 0:16, :], in0=L[:, :, 1:513], in1=L[:, :, 0:512],
               op=Alu.not_equal)
            tt(out=q2[:, 1:16, :], in0=L[:, 1:16, 1:513],
               in1=L[:, 0:15, 1:513], op=Alu.not_equal)
            tt(out=q2[:, 0:1, :], in0=L[:, 0:1, 1:513], in1=lg(0),
               op=Alu.not_equal)
            tt(out=q3[:, :, 1:513], in0=q1[:, 0:16, :], in1=q2[:, :, :],
               op=Alu.mult)
            tt(out=q1[:, 1:16, :], in0=L[:, 1:16, 1:513],
               in1=L[:, 0:15, 0:512], op=Alu.not_equal)
            tt(out=q1[:, 0:1, :], in0=L[:, 0:1, 1:513], in1=lg(-1),
               op=Alu.not_equal)
            tt(out=q4[:, 1:16, :], in0=L[:, 1:16, 1:513],
               in1=L[:, 0:15, 2:514], op=Alu.not_equal)
            tt(out=q4[:, 0:1, :], in0=L[:, 0:1, 1:513], in1=lg(+1),
               op=Alu.not_equal)
            tt(out=q2[:, :, :], in0=q1[:, 0:16, :], in1=q4[:, :, :],
               op=Alu.mult)
            tt(out=q4[:, :, :], in0=q3[:, :, 1:513], in1=q2[:, :, :],
               op=Alu.mult)
            V(vector.tensor_tensor(out=q2[:, :, :], in0=q4[:, :, :],
                                   in1=L[:, :, 1:513], op=Alu.mult),
              update=(s_dv, 1))

    return nc


def make_in_maps3(predict, target):
    """v3 layout: partition blocks ordered by DESCENDING seed base:
    [t_{2c+1}(+16H), t_{2c}(+16H), p_{2c+1}, p_{2c}]. Shifting the target
    seed space by +16*HW preserves its propagation (order-isomorphic) and
    distinct count, and makes every cross-image bottom-ghost leak smaller
    than any local label so max() discards it."""
    pu, pd = _build_perms()
    in_maps = []
    for c in range(N_CORES):
        imgs = [target[2 * c + 1, 0], target[2 * c, 0],
                predict[2 * c + 1, 0], predict[2 * c, 0]]
        xc = np.stack(imgs).astype(np.float32)
        xc = np.ascontiguousarray(xc.reshape(4, 32, 16, 512).reshape(128, 16, 512))
        bases = [(2 * c + 17) * HW_IMG, (2 * c + 16) * HW_IMG,
                 (2 * c + 1) * HW_IMG, (2 * c) * HW_IMG]
        cb = np.zeros((128, 1), dtype=np.float32)
        for j in range(4):
            cb[32 * j:32 * j + 32] = bases[j] - HW_IMG * j
        in_maps.append({
            "x": xc, "cbase": cb, "permup": pu, "permdn": pd,
            "thrin": np.zeros((128, 1), dtype=np.float32),
        })
    return in_maps


def _host_combine3(stats, cands):
    dices = []
    for c in range(N_CORES):
        st = stats[c].astype(np.float64)
        # image 2c+1: predict block2 (p 64:96) x target block0 (p 0:32)
        # image 2c  : predict block3 (p 96:128) x target block1 (p 32:64)
        for pp, tp in ((64, 0), (96, 32)):
            num = st[pp:pp + 32, 0].sum()
            den = st[pp:pp + 32, 1].sum() + st[tp:tp + 32, 1].sum()
            dices.append(1.0 - (num + 1.0) / (den + 1.0))
    mdice = np.mean(dices)

    pv = np.concatenate([cands[c][64:128].ravel() for c in range(N_CORES)])
    tv = np.concatenate([cands[c][0:64].ravel() for c in range(N_CORES)])
    pu_ = np.unique(pv[pv > 0]).size
    tu_ = np.unique(tv[tv > 0]).size
    pm = sum(stats[c][64:128, 2].sum(dtype=np.float64) for c in range(N_CORES))
    tm = sum(stats[c][0:64, 2].sum(dtype=np.float64) for c in range(N_CORES))
    total = 16 * HW_IMG
    nl = pu_ + (1 if pm < total else 0) - 1
    nt = tu_ + (1 if tm < total else 0)
    with np.errstate(divide="ignore", invalid="ignore"):
        pen = np.sqrt(np.float32(nl) / np.float32(nt))
    if not np.isfinite(pen):
        pen = np.float32(16.0)
    pen = np.clip(pen, np.float32(1.0), np.float32(16.0))
    return np.float32(np.float32(mdice) * pen)
